# revision 15
# baseline (speedup 1.0000x reference)
"""AUTKC loss kernel for Trainium2 (Bass/Tile), 8-core data-parallel.

Computes: p = softmax(pred, -1); exclude the positive class y per row;
top-(K+1)=6 negative probs; loss = mean_rows( sum_j (1 + p_j - p_y)^2 / K ).

Math notes:
  * softmax is shift-invariant; inputs are ~N(0,1) (|x| < ~7) so exp(x)
    never overflows fp32 -> skip the row-max pass entirely.  s = sum(exp(x)),
    p_i = exp(x_i)/s  exactly equals the reference softmax.
  * top-6 of p excluding index y  ==  top-6 of x with ONE occurrence of the
    VALUE x[y] removed (softmax is monotonic; equal values are
    interchangeable in the loss sum).  Implemented with the DVE Max8 +
    MatchReplace instructions.

Per core (256 rows x 50257 cols, fp32 = 51.5 MB):
  stream 2 row-blocks x 7 column tiles through SBUF;
  per tile: ACT exp+accumulate (row sums), DVE max8 (top-8 candidates);
  tail per row-block: combine candidates, drop the positive, exp the 7
  survivors, squared loss, [128,1] per-row loss -> DRAM.
Host: shard inputs, all-reduce (sum) the per-row losses, /(K*B).
"""

import os

import numpy as np

import concourse.bass as bass
import concourse.mybir as mybir
from concourse import bacc
from concourse.bass_utils import run_bass_kernel_spmd
from concourse.tile import TileContext

N_CORES = 8
B, C = 2048, 50257
K = 5
ROWS_PER_CORE = B // N_CORES  # 256
P = 128
N_RB = ROWS_PER_CORE // P  # 2 row blocks per core

# Column tiling: 7 near-equal tiles (each DMA ~3.7 MB, max8 free-size <= 16384)
N_CT = 7
_base = C // N_CT
_rem = C - _base * N_CT
WIDTHS = [_base + 1] * _rem + [_base] * (N_CT - _rem)
assert sum(WIDTHS) == C

BIG = 3.0e38  # sentinel: never matches data; replaced slots sort last


def _build_nc(repeat: int = 1, rows_per_core: int = ROWS_PER_CORE,
              n_cols: int = C, widths: list[int] | None = None) -> bass.Bass:
    """repeat>1 builds a benchmark variant that streams the same data
    `repeat` times (identical output, ~repeat x device work) so device
    exec time can be estimated by wall-clock differencing.
    rows_per_core/n_cols/widths are overridable for small CoreSim tests."""
    if widths is None:
        widths = WIDTHS if n_cols == C else [n_cols]
    assert sum(widths) == n_cols
    n_rb = rows_per_core // P
    n_ct = len(widths)

    nc = bacc.Bacc(None)
    pred = nc.declare_dram_parameter(
        "pred", [rows_per_core, n_cols], mybir.dt.float32, isOutput=False
    )
    # yflat[r] = r * n_cols + y[r]  (flat element index of the positive logit)
    yflat = nc.declare_dram_parameter(
        "yflat", [rows_per_core, 1], mybir.dt.uint32, isOutput=False
    )
    loss = nc.declare_dram_parameter(
        "loss", [n_rb, P], mybir.dt.float32, isOutput=True
    )

    pred_ap = pred[:, :]
    pred_rb = pred_ap.rearrange("(n p) c -> n p c", p=P)  # [n_rb, 128, C]
    pred_flat = pred_ap.rearrange("r (c o) -> (r c) o", o=1)  # [RPC*C, 1] for the gather
    y_rb = yflat[:, :].rearrange("(n p) o -> n p o", p=P)  # [n_rb, 128, 1]
    loss_rb = loss[:, :].rearrange("n (p o) -> n p o", o=1)  # [n_rb, 128, 1]

    with TileContext(nc) as tc:
        with (
            tc.tile_pool(name="data", bufs=4) as data_pool,
            tc.tile_pool(name="escr", bufs=2) as escr_pool,
            tc.tile_pool(name="stats", bufs=2) as stats_pool,
        ):
            for rb in [rb for _ in range(repeat) for rb in range(n_rb)]:
                # --- gather the positive logit x[r, y_r] for this row block
                idx = stats_pool.tile([P, 1], mybir.dt.uint32, tag="idx")
                nc.sync.dma_start(out=idx[:], in_=y_rb[rb])
                pos = stats_pool.tile([P, 1], mybir.dt.float32, tag="pos")
                nc.gpsimd.indirect_dma_start(
                    out=pos[:],
                    out_offset=None,
                    in_=pred_flat,
                    in_offset=bass.IndirectOffsetOnAxis(ap=idx[:, 0:1], axis=0),
                )

                # --- streaming pass over the row block
                cand = stats_pool.tile([P, 8 * n_ct], mybir.dt.float32, tag="cand")
                sums = stats_pool.tile([P, n_ct], mybir.dt.float32, tag="sums")
                col = 0
                for t, w in enumerate(widths):
                    data = data_pool.tile([P, w], mybir.dt.float32, tag="data")
                    nc.sync.dma_start(out=data[:], in_=pred_rb[rb][:, col : col + w])
                    # exp + per-row accumulate; the elementwise output is dead
                    # (bf16 scratch just to minimise SBUF write traffic)
                    escr = escr_pool.tile([P, w], mybir.dt.bfloat16, tag="escr")
                    nc.scalar.activation(
                        out=escr[:],
                        in_=data[:],
                        func=mybir.ActivationFunctionType.Exp,
                        accum_out=sums[:, t : t + 1],
                    )
                    # top-8 of this tile -> candidate pool
                    nc.vector.max(out=cand[:, 8 * t : 8 * (t + 1)], in_=data[:])
                    col += w

                # --- row-block tail (all [128, <=56] sized ops)
                s = stats_pool.tile([P, 1], mybir.dt.float32, tag="s")
                nc.vector.reduce_sum(s[:], sums[:], axis=mybir.AxisListType.X)
                rcp = stats_pool.tile([P, 1], mybir.dt.float32, tag="rcp")
                nc.vector.reciprocal(rcp[:], s[:])

                rep = stats_pool.tile([P, 8], mybir.dt.float32, tag="rep")
                nc.vector.memset(rep[:, 1:8], BIG)
                nc.vector.tensor_copy(rep[:, 0:1], pos[:])

                top8a = stats_pool.tile([P, 8], mybir.dt.float32, tag="top8a")
                nc.vector.max(out=top8a[:], in_=cand[:])
                # remove ONE occurrence of the positive value (if in top-8)
                top8c = stats_pool.tile([P, 8], mybir.dt.float32, tag="top8c")
                nc.vector.match_replace(
                    out=top8c[:], in_to_replace=rep[:], in_values=top8a[:],
                    imm_value=-BIG,
                )
                z = stats_pool.tile([P, 8], mybir.dt.float32, tag="z")
                nc.vector.max(out=z[:], in_=top8c[:])  # re-sort; cols 0..5 = top-6 negs
                nc.vector.tensor_copy(z[:, 6:7], pos[:])  # col 6 = x[y]

                ez = stats_pool.tile([P, 8], mybir.dt.float32, tag="ez")
                nc.scalar.activation(
                    out=ez[:, 0:7], in_=z[:, 0:7],
                    func=mybir.ActivationFunctionType.Exp,
                )
                # d = (e_j - e_pos) / s ; then 1 + d ; then sum of squares
                d1 = stats_pool.tile([P, 6], mybir.dt.float32, tag="d1")
                nc.vector.tensor_scalar(
                    out=d1[:], in0=ez[:, 0:6],
                    scalar1=ez[:, 6:7], scalar2=rcp[:, 0:1],
                    op0=mybir.AluOpType.subtract, op1=mybir.AluOpType.mult,
                )
                nc.vector.tensor_scalar(
                    out=d1[:], in0=d1[:], scalar1=1.0, scalar2=None,
                    op0=mybir.AluOpType.add,
                )
                # NOTE: tensor_tensor_reduce(accum_out=...) crashes the device
                # on this runtime build -- use mult + reduce_sum instead.
                sq = stats_pool.tile([P, 6], mybir.dt.float32, tag="sq")
                loss_row = stats_pool.tile([P, 1], mybir.dt.float32, tag="loss_row")
                nc.vector.tensor_mul(out=sq[:], in0=d1[:], in1=d1[:])
                nc.vector.reduce_sum(loss_row[:], sq[:], axis=mybir.AxisListType.X)
                nc.sync.dma_start(out=loss_rb[rb], in_=loss_row[:])
    nc.finalize()
    return nc


_CACHE: dict = {}


def _get_nc() -> bass.Bass:
    if "nc" not in _CACHE:
        _CACHE["nc"] = _build_nc()
    return _CACHE["nc"]


def kernel(pred, y, epoch=None, _trace=False, **_ignored) -> np.ndarray:
    pred = np.asarray(pred)
    assert pred.shape == (B, C) and pred.dtype == np.float32, (pred.shape, pred.dtype)
    y = np.asarray(y).astype(np.int64)

    in_maps = []
    row_ids = np.arange(ROWS_PER_CORE, dtype=np.int64)
    for c in range(N_CORES):
        r0 = c * ROWS_PER_CORE
        shard = np.ascontiguousarray(pred[r0 : r0 + ROWS_PER_CORE])
        yflat = (row_ids * C + y[r0 : r0 + ROWS_PER_CORE]).astype(np.uint32)
        in_maps.append({"pred": shard, "yflat": yflat.reshape(ROWS_PER_CORE, 1)})

    nc = _get_nc()
    try:
        res = run_bass_kernel_spmd(
            nc, in_maps, core_ids=list(range(N_CORES)), trace=_trace
        )
    except ModuleNotFoundError:
        # BASS_TRACE set but this container lacks the axon NTFF hook module;
        # retry with tracing force-disabled.
        os.environ["BASS_NEVER_TRACE"] = "1"
        res = run_bass_kernel_spmd(
            nc, in_maps, core_ids=list(range(N_CORES)), trace=False
        )
    _CACHE["last_results"] = res

    total = 0.0
    for r in res.results:
        total += r["loss"].astype(np.float64).sum()
    return np.asarray(total / (K * B), dtype=np.float32)
